# revision 12
# baseline (speedup 1.0000x reference)
"""AWQ linear kernel for Trainium2, 8-core tensor/data-parallel SPMD.

Computes out = x @ (weight * weight_scale).T + bias with
  x:[4,2048,4096] f32, weight:[4096,4096] int32 (int8-valued),
  weight_scale:[4096,1] f32, bias:[4096] f32.

Sharding: tokens (B*S=8192) split 2-way, out-features split 4-way
-> 8 cores, each computing a [4096, 1024] f32 output shard. No
cross-core communication.

Per-core plan (all engines overlapped by the Tile scheduler):
 - weight int32 -> SBUF, dequant+scale on ScalarE -> bf16 (ints <=126 are
   bf16-exact), bounce through a DRAM scratch, then DMA-xbar-transpose
   loads produce W^T [in,out] tiles in SBUF (resident, 8.4MB).
 - x f32 -> bf16 via SWDGE cast-DMA into a DRAM scratch (chunked by 512
   tokens), then DMA-xbar-transpose loads produce x^T [in,tok] tiles.
 - TensorE: 2048 bf16 matmuls [K=128,M=128]x[K=128,N=512] accumulating
   fp32 in PSUM over K=4096.
 - VectorE: psum + bias (pre-broadcast across partitions via a K=1
   fp32 matmul against ones) -> SBUF f32 -> DMA out.
"""

import contextlib

import numpy as np

import concourse.bass as bass
import concourse.tile as tile
import concourse.mybir as mybir
from concourse import bacc
from concourse.bass_utils import run_bass_kernel_spmd

P = 128

# full problem
B, S = 4, 2048
IN_F = 4096
OUT_F = 4096
TOK_SHARDS = 2   # token halves
OUT_SHARDS = 4   # out-feature quarters
N_CORES = TOK_SHARDS * OUT_SHARDS

# per-core shard
TOK = (B * S) // TOK_SHARDS     # 4096
OUTF = OUT_F // OUT_SHARDS      # 1024
CHUNK = 512                     # tokens per x pipeline chunk


def build_nc(tok=TOK, in_f=IN_F, outf=OUTF, chunk=CHUNK, x_mode="bitcast"):
    kc_n = in_f // P            # k chunks of 128
    nch = tok // chunk          # token chunks
    mb = chunk // P             # m blocks (128 tokens) per chunk
    nhw = min(512, outf)        # matmul free dim
    nnh = outf // nhw           # n tiles per output row block
    wrow_n = outf // P          # weight row chunks of 128
    wcc = 2048                  # weight prep column chunk
    wcc_n = in_f // wcc if in_f >= wcc else 1
    wcc = in_f // wcc_n

    nc = bacc.Bacc("TRN2", target_bir_lowering=False, debug=False,
                   num_devices=N_CORES)
    x_h = nc.dram_tensor("x", [tok, in_f], mybir.dt.float32,
                         kind="ExternalInput").ap()
    w_h = nc.dram_tensor("weight", [outf, in_f], mybir.dt.int32,
                         kind="ExternalInput").ap()
    ws_h = nc.dram_tensor("weight_scale", [outf, 1], mybir.dt.float32,
                          kind="ExternalInput").ap()
    b_h = nc.dram_tensor("bias", [1, outf], mybir.dt.float32,
                         kind="ExternalInput").ap()
    out_h = nc.dram_tensor("out", [tok, outf], mybir.dt.float32,
                           kind="ExternalOutput").ap()

    with tile.TileContext(nc) as tc, contextlib.ExitStack() as ctx:
        dram_pool = ctx.enter_context(tc.tile_pool(name="dram", bufs=1, space="DRAM"))
        wt_pool = ctx.enter_context(tc.tile_pool(name="wt", bufs=1))
        const_pool = ctx.enter_context(tc.tile_pool(name="const", bufs=1))
        wprep_pool = ctx.enter_context(tc.tile_pool(name="wprep", bufs=2))
        xt_pool = ctx.enter_context(tc.tile_pool(name="xt", bufs=2))
        out_pool = ctx.enter_context(tc.tile_pool(name="outp", bufs=3))
        psum_pool = ctx.enter_context(tc.tile_pool(name="psum", bufs=4, space="PSUM"))

        x_bf_dram = dram_pool.tile([tok, in_f], mybir.dt.bfloat16)
        w_bf_dram = dram_pool.tile([outf, in_f], mybir.dt.bfloat16)

        # bias broadcast across partitions: psum[p, n] = ones[1, p].T @ bias[1, n]
        bias_sb = const_pool.tile([1, outf], mybir.dt.float32)
        nc.scalar.dma_start(bias_sb, b_h)
        ones = const_pool.tile([1, P], mybir.dt.float32)
        nc.vector.memset(ones, 1.0)
        bias_rep = const_pool.tile([P, outf], mybir.dt.float32)
        for nh in range(nnh):
            pb = psum_pool.tile([P, nhw], mybir.dt.float32, tag="ps")
            nc.tensor.matmul(pb, ones, bias_sb[:, nh * nhw:(nh + 1) * nhw],
                             start=True, stop=True)
            nc.vector.tensor_copy(out=bias_rep[:, nh * nhw:(nh + 1) * nhw], in_=pb)

        # x cast f32 -> bf16 (SWDGE casts inline); first two chunks up front,
        # the rest just-in-time inside the main loop (prefetch distance 2).
        def cast_chunk(c):
            nc.gpsimd.dma_start(x_bf_dram[c * chunk:(c + 1) * chunk, :],
                                x_h[c * chunk:(c + 1) * chunk, :])

        for c in range(min(2, nch)):
            cast_chunk(c)

        # W dequant: int32 * scale -> bf16, bounced via DRAM for xbar
        # transpose. Column-chunk outer so the first W^T transposes (low kc)
        # unblock as early as possible.
        scs = []
        for wc in range(wrow_n):
            sc = wprep_pool.tile([P, 1], mybir.dt.float32, tag="sc", bufs=wrow_n)
            nc.scalar.dma_start(sc, ws_h[wc * P:(wc + 1) * P, :])
            scs.append(sc)
        for cc in range(wcc_n):
            for wc in range(wrow_n):
                w_i32 = wprep_pool.tile([P, wcc], mybir.dt.int32, tag="wi32")
                nc.scalar.dma_start(w_i32, w_h[wc * P:(wc + 1) * P,
                                               cc * wcc:(cc + 1) * wcc])
                w_bf = wprep_pool.tile([P, wcc], mybir.dt.bfloat16, tag="wbf")
                nc.scalar.mul(w_bf, w_i32, scs[wc])
                nc.scalar.dma_start(w_bf_dram[wc * P:(wc + 1) * P,
                                              cc * wcc:(cc + 1) * wcc], w_bf)

        # W^T tiles, resident: wt[p, kc, :] = W_bf[:, kc*128 + p]
        # Alternate the two HWDGE queues so transpose issue isn't serialized
        # on one engine.
        wt = wt_pool.tile([P, kc_n, outf], mybir.dt.bfloat16)
        for kc in range(kc_n):
            nc.sync.dma_start(wt[:, kc, :], w_bf_dram[:, kc * P:(kc + 1) * P],
                              transpose=True)

        # main pipeline over token chunks
        for c in range(nch):
            xt = xt_pool.tile([P, kc_n, chunk], mybir.dt.bfloat16, tag="xt")
            for kc in range(kc_n):
                nc.sync.dma_start(xt[:, kc, :],
                                  x_bf_dram[c * chunk:(c + 1) * chunk,
                                            kc * P:(kc + 1) * P],
                                  transpose=True)
            if c + 2 < nch:
                cast_chunk(c + 2)
            for m in range(mb):
                out_sb = out_pool.tile([P, outf], mybir.dt.float32, tag="osb")
                for nh in range(nnh):
                    ps = psum_pool.tile([P, nhw], mybir.dt.float32, tag="ps")
                    for kc in range(kc_n):
                        nc.tensor.matmul(
                            ps,
                            xt[:, kc, m * P:(m + 1) * P],
                            wt[:, kc, nh * nhw:(nh + 1) * nhw],
                            start=(kc == 0), stop=(kc == kc_n - 1))
                    nc.vector.tensor_add(out=out_sb[:, nh * nhw:(nh + 1) * nhw],
                                         in0=ps,
                                         in1=bias_rep[:, nh * nhw:(nh + 1) * nhw])
                row0 = (c * mb + m) * P
                # out-stores ride the SWDGE queue: the HWDGE queues carry the
                # latency-critical transposes and issue strictly in order.
                nc.gpsimd.dma_start(out_h[row0:row0 + P, :], out_sb)
    nc.compile()
    return nc


def shard_inputs(x, weight, weight_scale, bias):
    xf = np.ascontiguousarray(x.reshape(B * S, IN_F))
    in_maps = []
    for core in range(N_CORES):
        r, q = divmod(core, OUT_SHARDS)
        in_maps.append({
            "x": np.ascontiguousarray(xf[r * TOK:(r + 1) * TOK]),
            "weight": np.ascontiguousarray(weight[q * OUTF:(q + 1) * OUTF]),
            "weight_scale": np.ascontiguousarray(weight_scale[q * OUTF:(q + 1) * OUTF]),
            "bias": np.ascontiguousarray(bias[q * OUTF:(q + 1) * OUTF]).reshape(1, OUTF),
        })
    return in_maps


def gather_outputs(results):
    halves = []
    for r in range(TOK_SHARDS):
        quarters = [results[r * OUT_SHARDS + q]["out"] for q in range(OUT_SHARDS)]
        halves.append(np.concatenate(quarters, axis=1))
    full = np.concatenate(halves, axis=0)
    return np.ascontiguousarray(full.reshape(B, S, OUT_F).astype(np.float32))


_NC_CACHE = {}

X_MODE = "scratch"


def _get_nc(x_mode=None):
    x_mode = x_mode or X_MODE
    if x_mode not in _NC_CACHE:
        _NC_CACHE[x_mode] = build_nc(x_mode=x_mode)
    return _NC_CACHE[x_mode]


def kernel(x, weight, weight_scale, bias, _trace=False, _x_mode=None):
    nc = _get_nc(_x_mode)
    in_maps = shard_inputs(np.asarray(x), np.asarray(weight),
                           np.asarray(weight_scale), np.asarray(bias))
    res = run_bass_kernel_spmd(nc, in_maps, core_ids=list(range(N_CORES)),
                               trace=_trace)
    out = gather_outputs(res.results)
    if _trace:
        return out, res
    return out


# revision 15
# speedup vs baseline: 1.0747x; 1.0747x over previous
"""AWQ linear kernel for Trainium2, 8-core tensor/data-parallel SPMD.

Computes out = x @ (weight * weight_scale).T + bias with
  x:[4,2048,4096] f32, weight:[4096,4096] int32 (int8-valued),
  weight_scale:[4096,1] f32, bias:[4096] f32.

Sharding: tokens (B*S=8192) split 2-way, out-features split 4-way
-> 8 cores, each computing a [4096, 1024] f32 output shard. No
cross-core communication.

Per-core plan (all engines overlapped by the Tile scheduler):
 - weight int32 -> SBUF, dequant+scale on ScalarE -> bf16 (ints <=126 are
   bf16-exact), bounce through a DRAM scratch, then DMA-xbar-transpose
   loads produce W^T [in,out] tiles in SBUF (resident, 8.4MB).
 - x f32 -> bf16 via SWDGE cast-DMA into a DRAM scratch (chunked by 512
   tokens), then DMA-xbar-transpose loads produce x^T [in,tok] tiles.
 - TensorE: 2048 bf16 matmuls [K=128,M=128]x[K=128,N=512] accumulating
   fp32 in PSUM over K=4096.
 - VectorE: psum + bias (pre-broadcast across partitions via a K=1
   fp32 matmul against ones) -> SBUF f32 -> DMA out.
"""

import contextlib

import numpy as np

import concourse.bass as bass
import concourse.tile as tile
import concourse.mybir as mybir
from concourse import bacc
from concourse.bass_utils import run_bass_kernel_spmd

P = 128

# full problem
B, S = 4, 2048
IN_F = 4096
OUT_F = 4096
TOK_SHARDS = 2   # token halves
OUT_SHARDS = 4   # out-feature quarters
N_CORES = TOK_SHARDS * OUT_SHARDS

# per-core shard
TOK = (B * S) // TOK_SHARDS     # 4096
OUTF = OUT_F // OUT_SHARDS      # 1024
CHUNK = 512                     # tokens per x pipeline chunk


def build_nc(tok=TOK, in_f=IN_F, outf=OUTF, chunk=CHUNK, x_mode="bitcast"):
    kc_n = in_f // P            # k chunks of 128
    nch = tok // chunk          # token chunks
    mb = chunk // P             # m blocks (128 tokens) per chunk
    nhw = min(512, outf)        # matmul free dim
    nnh = outf // nhw           # n tiles per output row block
    wrow_n = outf // P          # weight row chunks of 128
    wcc = 2048                  # weight prep column chunk
    wcc_n = in_f // wcc if in_f >= wcc else 1
    wcc = in_f // wcc_n

    nc = bacc.Bacc("TRN2", target_bir_lowering=False, debug=False,
                   num_devices=N_CORES)
    x_h = nc.dram_tensor("x", [tok, in_f], mybir.dt.float32,
                         kind="ExternalInput").ap()
    w_h = nc.dram_tensor("weight", [outf, in_f], mybir.dt.int32,
                         kind="ExternalInput").ap()
    ws_h = nc.dram_tensor("weight_scale", [outf, 1], mybir.dt.float32,
                          kind="ExternalInput").ap()
    b_h = nc.dram_tensor("bias", [1, outf], mybir.dt.float32,
                         kind="ExternalInput").ap()
    out_h = nc.dram_tensor("out", [tok, outf], mybir.dt.float32,
                           kind="ExternalOutput").ap()

    with tile.TileContext(nc) as tc, contextlib.ExitStack() as ctx:
        dram_pool = ctx.enter_context(tc.tile_pool(name="dram", bufs=1, space="DRAM"))
        wt_pool = ctx.enter_context(tc.tile_pool(name="wt", bufs=1))
        const_pool = ctx.enter_context(tc.tile_pool(name="const", bufs=1))
        wprep_pool = ctx.enter_context(tc.tile_pool(name="wprep", bufs=2))
        xt_pool = ctx.enter_context(tc.tile_pool(name="xt", bufs=2))
        out_pool = ctx.enter_context(tc.tile_pool(name="outp", bufs=3))
        psum_pool = ctx.enter_context(tc.tile_pool(name="psum", bufs=4, space="PSUM"))

        x_bf_dram = dram_pool.tile([tok, in_f], mybir.dt.bfloat16)

        # bias broadcast across partitions: psum[p, n] = ones[1, p].T @ bias[1, n]
        bias_sb = const_pool.tile([1, outf], mybir.dt.float32)
        nc.scalar.dma_start(bias_sb, b_h)
        ones = const_pool.tile([1, P], mybir.dt.float32)
        nc.vector.memset(ones, 1.0)
        bias_rep = const_pool.tile([P, outf], mybir.dt.float32)
        for nh in range(nnh):
            pb = psum_pool.tile([P, nhw], mybir.dt.float32, tag="ps")
            nc.tensor.matmul(pb, ones, bias_sb[:, nh * nhw:(nh + 1) * nhw],
                             start=True, stop=True)
            nc.vector.tensor_copy(out=bias_rep[:, nh * nhw:(nh + 1) * nhw], in_=pb)

        # x cast f32 -> bf16 (SWDGE casts inline), all queued up front on the
        # gpsimd ring; the DMA engines round-robin them against other queues.
        for c in range(nch):
            nc.gpsimd.dma_start(x_bf_dram[c * chunk:(c + 1) * chunk, :],
                                x_h[c * chunk:(c + 1) * chunk, :])

        # W path, all on-chip (PE is idle during the head anyway):
        # int32 load -> ACT dequant+scale -> bf16 -> PE transpose via
        # identity -> DVE copy PSUM -> resident W^T. No DRAM bounce, and the
        # sync HWDGE queue stays dedicated to the x transposes.
        ident = const_pool.tile([P, P], mybir.dt.bfloat16)
        from concourse.masks import make_identity
        make_identity(nc, ident)

        wt = wt_pool.tile([P, kc_n, outf], mybir.dt.bfloat16)
        scs = []
        for wc in range(wrow_n):
            sc = wprep_pool.tile([P, 1], mybir.dt.float32, tag="sc", bufs=wrow_n)
            nc.scalar.dma_start(sc, ws_h[wc * P:(wc + 1) * P, :])
            scs.append(sc)
        TGRP = 4  # transposes batched per psum tile
        for wc in range(wrow_n):
            for cc in range(wcc_n):
                w_i32 = wprep_pool.tile([P, wcc], mybir.dt.int32, tag="wi32")
                nc.scalar.dma_start(w_i32, w_h[wc * P:(wc + 1) * P,
                                               cc * wcc:(cc + 1) * wcc])
                w_bf = wprep_pool.tile([P, wcc], mybir.dt.bfloat16, tag="wbf")
                nc.scalar.mul(w_bf, w_i32, scs[wc])
                kc0 = cc * (wcc // P)
                for g in range(wcc // P // TGRP):
                    ptr = psum_pool.tile([P, TGRP * P], mybir.dt.bfloat16,
                                         tag="ptr", bufs=2)
                    for j in range(TGRP):
                        nc.tensor.transpose(
                            ptr[:, j * P:(j + 1) * P],
                            w_bf[:, (g * TGRP + j) * P:(g * TGRP + j + 1) * P],
                            ident)
                    kta = kc0 + g * TGRP
                    nc.vector.tensor_copy(
                        out=wt[:, kta:kta + TGRP, wc * P:(wc + 1) * P],
                        in_=ptr.rearrange("p (t q) -> p t q", t=TGRP))

        # main pipeline over token chunks
        for c in range(nch):
            xt = xt_pool.tile([P, kc_n, chunk], mybir.dt.bfloat16, tag="xt")
            for kc in range(kc_n):
                nc.sync.dma_start(xt[:, kc, :],
                                  x_bf_dram[c * chunk:(c + 1) * chunk,
                                            kc * P:(kc + 1) * P],
                                  transpose=True)
            for m in range(mb):
                out_sb = out_pool.tile([P, outf], mybir.dt.float32, tag="osb")
                for nh in range(nnh):
                    ps = psum_pool.tile([P, nhw], mybir.dt.float32, tag="ps")
                    for kc in range(kc_n):
                        nc.tensor.matmul(
                            ps,
                            xt[:, kc, m * P:(m + 1) * P],
                            wt[:, kc, nh * nhw:(nh + 1) * nhw],
                            start=(kc == 0), stop=(kc == kc_n - 1))
                    nc.vector.tensor_add(out=out_sb[:, nh * nhw:(nh + 1) * nhw],
                                         in0=ps,
                                         in1=bias_rep[:, nh * nhw:(nh + 1) * nhw])
                row0 = (c * mb + m) * P
                # out-stores ride the SWDGE queue: the HWDGE queues carry the
                # latency-critical transposes and issue strictly in order.
                nc.gpsimd.dma_start(out_h[row0:row0 + P, :], out_sb)
    nc.compile()
    return nc


def shard_inputs(x, weight, weight_scale, bias):
    xf = np.ascontiguousarray(x.reshape(B * S, IN_F))
    in_maps = []
    for core in range(N_CORES):
        r, q = divmod(core, OUT_SHARDS)
        in_maps.append({
            "x": np.ascontiguousarray(xf[r * TOK:(r + 1) * TOK]),
            "weight": np.ascontiguousarray(weight[q * OUTF:(q + 1) * OUTF]),
            "weight_scale": np.ascontiguousarray(weight_scale[q * OUTF:(q + 1) * OUTF]),
            "bias": np.ascontiguousarray(bias[q * OUTF:(q + 1) * OUTF]).reshape(1, OUTF),
        })
    return in_maps


def gather_outputs(results):
    halves = []
    for r in range(TOK_SHARDS):
        quarters = [results[r * OUT_SHARDS + q]["out"] for q in range(OUT_SHARDS)]
        halves.append(np.concatenate(quarters, axis=1))
    full = np.concatenate(halves, axis=0)
    return np.ascontiguousarray(full.reshape(B, S, OUT_F).astype(np.float32))


_NC_CACHE = {}

X_MODE = "scratch"


def _get_nc(x_mode=None):
    x_mode = x_mode or X_MODE
    if x_mode not in _NC_CACHE:
        _NC_CACHE[x_mode] = build_nc(x_mode=x_mode)
    return _NC_CACHE[x_mode]


def kernel(x, weight, weight_scale, bias, _trace=False, _x_mode=None):
    nc = _get_nc(_x_mode)
    in_maps = shard_inputs(np.asarray(x), np.asarray(weight),
                           np.asarray(weight_scale), np.asarray(bias))
    res = run_bass_kernel_spmd(nc, in_maps, core_ids=list(range(N_CORES)),
                               trace=_trace)
    out = gather_outputs(res.results)
    if _trace:
        return out, res
    return out
